# revision 1
# baseline (speedup 1.0000x reference)
"""ConvLSTM block (B=16, T=16, 32->64ch, 64x64, 3x3 SAME conv) on 8 TRN2 cores.

Strategy: data-parallel over batch (2 images/core). All conv operands are
fp16 (moving-operand rate on the PE is 1 row/cycle, same as f32r, at half
the bytes; end-to-end rel err ~7e-4). The 3x3 conv over concat([x_t, h])
is computed as 7 K-packed matmuls per output tile instead of 9 K=96 ones:
shifted copies of the input planes are partition-stacked so each matmul
contracts over up to 128 rows:

  S1-S3 (ky=0..2): Buf1 = [x | x@(0,1) | h], window (y0+ky, 0)
                   -> taps x(ky,0), x(ky,1), h(ky,0)
  S4-S6 (ky=0..2): Buf2 = [h | h@(0,1)], window (y0+ky, 1)
                   -> taps h(ky,1), h(ky,2)
  S7:              Buf3 = [x@r0 | x@r1 | x@r2], window (y0, 2)
                   -> taps x(0,2), x(1,2), x(2,2)

Gate math: chunkA = [i;f] (one full-width sigmoid), chunkB = [g;o'] where
o' = tanh(o_raw/2) (weights/bias pre-halved) so one full-width tanh covers
both; sigma(z) = 0.5*tanh(z/2)+0.5 is recovered by writing v = 2h =
(o'+1)*tanh(c) into the h-planes with all h-columns of W pre-halved.
h-plane writes go directly from the compute engines (DVE + GPSIMD) into
the next step's conv buffers; the h@(0,0) copy for Buf2 is one contiguous
row-range DMA. Step 0 skips the h-only streams S4-S6 (h == 0).
"""

from contextlib import ExitStack

import numpy as np

import concourse.mybir as mybir
import concourse.tile as tile
from concourse import bacc
from concourse.bass_utils import run_bass_kernel_spmd

F32 = mybir.dt.float32
F16 = mybir.dt.float16
AF = mybir.ActivationFunctionType
ALU = mybir.AluOpType

# Problem shapes (hardcoded per harness contract).
B, T, CIN, HID, H, W = 16, 16, 32, 64, 64, 64
NCORES = 8
BL = B // NCORES            # images per core
CH = CIN + HID              # conv input channels
PH, PW = H + 2, W + 2       # zero-padded plane
RG_ROWS = 8                 # output rows per PSUM tile (8*64 = 512 = one bank)
NRG = H // RG_ROWS
NSTREAM = 7


def _build(steps=T):
    nc = bacc.Bacc("TRN2", target_bir_lowering=False, debug=False)
    x_d = nc.dram_tensor("xin", [BL, T, CIN, H, W], F32, kind="ExternalInput")
    w_d = nc.dram_tensor("win", [128, NSTREAM * 2 * 128], F16, kind="ExternalInput")
    b_d = nc.dram_tensor("bin", [128, 2], F32, kind="ExternalInput")
    o_d = nc.dram_tensor("out", [BL, HID, H, W], F32, kind="ExternalOutput")

    with tile.TileContext(nc) as tc:
        with ExitStack() as ctx:
            const = ctx.enter_context(tc.tile_pool(name="const", bufs=1))
            psum = ctx.enter_context(tc.tile_pool(name="psum", bufs=4, space="PSUM"))
            gp = ctx.enter_context(tc.tile_pool(name="gates", bufs=3))

            wsb = const.tile([128, NSTREAM * 2 * 128], F16, tag="wsb")
            nc.sync.dma_start(out=wsb[:, :], in_=w_d[:, :])
            bsb = const.tile([128, 2], F32, tag="bsb")
            nc.sync.dma_start(out=bsb[:, :], in_=b_d[:, :])

            # Ping-pong fp16 plane sets. buf1: x@(0,0) in [0:32), x@(0,1) in
            # [32:64), h@(0,0) in [64:128). buf2: h@(0,0) in [0:64),
            # h@(0,1) in [64:128). buf3: x row-shifted by 0/1/2.
            buf1 = [const.tile([128, BL, PH, PW], F16, tag=f"b1_{i}", name=f"b1_{i}")
                    for i in range(2)]
            buf2 = [const.tile([128, BL, PH, PW], F16, tag=f"b2_{i}", name=f"b2_{i}")
                    for i in range(2)]
            buf3 = [const.tile([96, BL, PH, PW], F16, tag=f"b3_{i}", name=f"b3_{i}")
                    for i in range(2)]
            for pb in buf1 + buf2 + buf3:
                nc.gpsimd.memset(pb[:, :, :, :], 0.0)
            # Cell state in partitions [64,128) (lane-aligned with f/o').
            cst = const.tile([128, BL, H * W], F16, tag="cst")
            nc.vector.memset(cst[:, :, :], 0.0)
            # f32 staging for x_t (GPSIMD converts to fp16 while scattering).
            xstage = const.tile([CIN, BL, H, W], F32, tag="xstage")

            for t in range(steps):
                cur1, nxt1 = buf1[t % 2], buf1[(t + 1) % 2]
                cur2, nxt2 = buf2[t % 2], buf2[(t + 1) % 2]
                cur3 = buf3[t % 2]

                # Stage x_t: HBM f32 -> fp16 padded plane + shifted copies.
                for img in range(BL):
                    nc.sync.dma_start(
                        out=xstage[:, img, :, :], in_=x_d[img, t, :, :, :]
                    )
                nc.gpsimd.tensor_copy(
                    cur1[0:CIN, :, 1 : H + 1, 1 : W + 1], xstage[:, :, :, :]
                )
                # x@(0,1): P2x[:, c] = P[:, c+1]
                nc.sync.dma_start(
                    out=cur1[32:64, :, :, 0 : PW - 1], in_=cur1[0:32, :, :, 1:PW]
                )
                # x row-shifted: grp_k rows [0 : PH-k) = P rows [k : PH)
                for k in range(3):
                    nc.sync.dma_start(
                        out=cur3[32 * k : 32 * k + 32, :, 0 : PH - k, :],
                        in_=cur1[0:32, :, k:PH, :],
                    )

                streams = list(range(NSTREAM)) if t > 0 else [0, 1, 2, 6]
                for img in range(BL):
                    for rg in range(NRG):
                        y0 = rg * RG_ROWS
                        ps = [
                            psum.tile([128, RG_ROWS, 64], F32, tag=f"ps{c}", name=f"ps{c}")
                            for c in range(2)
                        ]
                        for c in range(2):
                            for si, s in enumerate(streams):
                                if s < 3:
                                    k_sz = 128
                                    rhs = cur1[0:128, img, y0 + s : y0 + s + RG_ROWS, 0:64]
                                elif s < 6:
                                    k_sz = 128
                                    ky = s - 3
                                    rhs = cur2[0:128, img, y0 + ky : y0 + ky + RG_ROWS, 1:65]
                                else:
                                    k_sz = 96
                                    rhs = cur3[0:96, img, y0 : y0 + RG_ROWS, 2:66]
                                nc.tensor.matmul(
                                    out=ps[c][:, :, :],
                                    lhsT=wsb[0:k_sz, (s * 2 + c) * 128 : (s * 2 + c + 1) * 128],
                                    rhs=rhs,
                                    start=(si == 0),
                                    stop=(si == len(streams) - 1),
                                )

                        csl = cst[64:128, img, y0 * 64 : (y0 + RG_ROWS) * 64]

                        # chunkA = [i; f]: one full-width sigmoid.
                        sif = gp.tile([128, RG_ROWS, 64], F16, tag="sif")
                        nc.scalar.activation(
                            out=sif[:, :, :], in_=ps[0][:, :, :],
                            func=AF.Sigmoid, bias=bsb[:, 0:1],
                        )
                        # chunkB = [g; o']: one full-width tanh.
                        tgo = gp.tile([128, RG_ROWS, 64], F16, tag="tgo")
                        nc.scalar.activation(
                            out=tgo[:, :, :], in_=ps[1][:, :, :],
                            func=AF.Tanh, bias=bsb[:, 1:2],
                        )

                        # c = f*c + i*g with one cross-half DMA bridge.
                        pr1 = gp.tile([128, RG_ROWS, 64], F16, tag="pr1")
                        nc.vector.tensor_mul(pr1[0:64], sif[0:64], tgo[0:64])
                        nc.sync.dma_start(out=pr1[64:128], in_=pr1[0:64])
                        tmp = gp.tile([128, RG_ROWS, 64], F16, tag="tmp")
                        nc.vector.tensor_mul(tmp[64:128], sif[64:128], csl)
                        nc.vector.tensor_add(csl, tmp[64:128], pr1[64:128])
                        tct = gp.tile([128, RG_ROWS, 64], F16, tag="tct")
                        nc.scalar.activation(tct[64:128], csl, func=AF.Tanh)

                        if t < steps - 1:
                            # v = 2h = (o'+1)*tanh(c), written straight into
                            # the next step's conv planes (h-cols of W are
                            # pre-halved to compensate).
                            nc.vector.scalar_tensor_tensor(
                                out=nxt1[64:128, img, y0 + 1 : y0 + 9, 1 : W + 1],
                                in0=tgo[64:128], scalar=1.0, in1=tct[64:128],
                                op0=ALU.add, op1=ALU.mult,
                            )
                            nc.gpsimd.tensor_copy(
                                nxt2[64:128, img, y0 + 1 : y0 + 9, 0:W],
                                nxt1[64:128, img, y0 + 1 : y0 + 9, 1 : W + 1],
                            )
                            # h@(0,0) copy for buf2: contiguous row range.
                            nc.sync.dma_start(
                                out=nxt2[0:64, img, y0 + 1 : y0 + 9, :],
                                in_=nxt1[64:128, img, y0 + 1 : y0 + 9, :],
                            )
                        else:
                            # h = (0.5*o' + 0.5)*tanh(c); out = max(h, 0.01h)
                            e1 = gp.tile([128, RG_ROWS, 64], F32, tag="e1")
                            nc.vector.scalar_tensor_tensor(
                                out=e1[64:128], in0=tgo[64:128], scalar=0.5,
                                in1=tct[64:128], op0=ALU.mult, op1=ALU.mult,
                            )
                            ht = gp.tile([128, RG_ROWS, 64], F32, tag="ht")
                            nc.vector.scalar_tensor_tensor(
                                out=ht[64:128], in0=tct[64:128], scalar=0.5,
                                in1=e1[64:128], op0=ALU.mult, op1=ALU.add,
                            )
                            ost = gp.tile([128, RG_ROWS, 64], F32, tag="ost")
                            nc.vector.scalar_tensor_tensor(
                                out=ost[64:128], in0=ht[64:128], scalar=0.01,
                                in1=ht[64:128], op0=ALU.mult, op1=ALU.max,
                            )
                            nc.sync.dma_start(
                                out=o_d[img, :, y0 : y0 + RG_ROWS, :],
                                in_=ost[64:128, :, :],
                            )
    nc.compile()
    return nc


def _prep_weights(Wf, bf):
    Wp = np.asarray(Wf, np.float32).copy()     # [256, CH, 3, 3], gates [i,f,o,g]
    bp = np.asarray(bf, np.float32).copy()
    # h is fed as v = 2h: halve all h-columns.
    Wp[:, CIN:CH] *= 0.5
    # o' = tanh(o_raw/2): halve the o-gate rows and bias.
    Wp[128:192] *= 0.5
    bp = bp.copy()
    bp[128:192] *= 0.5
    # chunkA = [i; f], chunkB = [g; o'].
    chA = Wp[0:128]
    chB = np.concatenate([Wp[192:256], Wp[128:192]], axis=0)
    bA = bp[0:128]
    bB = np.concatenate([bp[192:256], bp[128:192]], axis=0)

    # wl[k, s, c, m]: stream s, chunk c, stationary column m, contraction k.
    wl = np.zeros((128, NSTREAM, 2, 128), np.float32)
    for c, Wc in enumerate([chA, chB]):
        for ky in range(3):                     # S1-S3: x(ky,0), x(ky,1), h(ky,0)
            wl[0:32, ky, c] = Wc[:, 0:CIN, ky, 0].T
            wl[32:64, ky, c] = Wc[:, 0:CIN, ky, 1].T
            wl[64:128, ky, c] = Wc[:, CIN:CH, ky, 0].T
        for ky in range(3):                     # S4-S6: h(ky,1), h(ky,2)
            wl[0:64, 3 + ky, c] = Wc[:, CIN:CH, ky, 1].T
            wl[64:128, 3 + ky, c] = Wc[:, CIN:CH, ky, 2].T
        for k in range(3):                      # S7: x(k,2)
            wl[32 * k : 32 * k + 32, 6, c] = Wc[:, 0:CIN, k, 2].T
    wl = np.ascontiguousarray(
        wl.reshape(128, NSTREAM * 2 * 128), dtype=np.float16
    )
    b2 = np.ascontiguousarray(np.stack([bA, bB], axis=1))  # [128, 2] f32
    return wl, b2


_NC_CACHE = {}


def _get_nc():
    if "nc" not in _NC_CACHE:
        _NC_CACHE["nc"] = _build()
    return _NC_CACHE["nc"]


def _in_maps(x, Wf, bf):
    x = np.ascontiguousarray(np.asarray(x, np.float32))
    wl, b2 = _prep_weights(Wf, bf)
    return [
        {
            "xin": np.ascontiguousarray(x[i * BL : (i + 1) * BL]),
            "win": wl,
            "bin": b2,
        }
        for i in range(NCORES)
    ]


def _run(x, W, b, trace=False, **spmd_kwargs):
    nc = _get_nc()
    res = run_bass_kernel_spmd(
        nc, _in_maps(x, W, b), core_ids=list(range(NCORES)), trace=trace,
        **spmd_kwargs,
    )
    out = np.concatenate([res.results[i]["out"] for i in range(NCORES)], axis=0)
    return np.ascontiguousarray(out, dtype=np.float32), res


def kernel(x, W, b):
    out, _ = _run(x, W, b)
    return out



# revision 16
# speedup vs baseline: 1.4623x; 1.4623x over previous
"""ConvLSTM block (B=16, T=16, 32->64ch, 64x64, 3x3 SAME) on 8 TRN2 cores.

Data-parallel over batch (2 images/core). The conv is computed with fp8-e4m3
DoubleRow matmuls (cost-model rate 0.5 cyc/row, K<=256/pass): per output tile
5 DR passes cover all 864 contraction rows via 9 "tap slots" — each slot
reads the SAME stored planes at a different (ky,kx) window offset, so h needs
no shifted copies at all.  Plane tile T[img] = [128, 66, 66] fp8: img0 has
x-channels on partitions 0:32 and h on 64:128 (img1 mirrored: h on 0:64) so
the next step's h can be written in-place by the vector engines.  x arrives
pre-quantized/padded from the host.  Weights are scaled by S=64 and
quantized to e4m3 on the host; the activation instruction un-scales.

Gate math: img0 chunkA=[i;f] (one sigmoid per 4-rg group), chunkB=[g;o'],
o'=tanh(o/2) with o-rows pre-halved; img1 uses swapped chunks ([f;i],[o';g])
so the c/h elementwise chain runs with both images packed on complementary
partition halves wherever the dataflow allows.  h is fed back as v=2h=
(o'+1)*tanh(c) with W h-columns pre-halved.
"""

from contextlib import ExitStack

import numpy as np
import ml_dtypes
import bass_rust

import concourse.mybir as mybir
import concourse.tile as tile
from concourse import bacc
from concourse.bass_utils import run_bass_kernel_spmd

F32 = mybir.dt.float32
F16 = mybir.dt.float16
F8 = mybir.dt.float8e4
AF = mybir.ActivationFunctionType
ALU = mybir.AluOpType
DR = mybir.MatmulPerfMode.DoubleRow

NP8 = ml_dtypes.float8_e4m3  # container dtype bass2jax expects for F8

# Problem shapes (hardcoded per harness contract).
B, T, CIN, HID, H, W = 16, 16, 32, 64, 64, 64
NCORES = 8
BL = B // NCORES            # images per core
PH = PW = 66                # padded plane
SCL = 64.0                  # weight scale (e4m3 range)
XRS = 64.0                  # x-residual plane scale
NPASS = 9
RG = 8                      # rows per PSUM bank tile
HROWS = 32                  # rows per half-group (4 rg)
HFREE = HROWS * W           # 2048

# 9 DR passes; HW requires ktile strides that are multiples of the row pitch,
# so ktiles pair taps within a column.  Each entry: ((tap_j0, kind_j0),
# (tap_j1, kind_j1)) with kind "A" = base weights, "R" = residual weights.
_PASS_TAPS = [
    (((0, 0), "A"), ((1, 0), "A")), (((0, 1), "A"), ((1, 1), "A")),
    (((0, 2), "A"), ((1, 2), "A")),
    (((1, 0), "R"), ((2, 0), "A")), (((1, 1), "R"), ((2, 1), "A")),
    (((1, 2), "R"), ((2, 2), "A")),
    (((0, 0), "R"), ((2, 0), "R")), (((0, 1), "R"), ((2, 1), "R")),
    (((0, 2), "R"), ((2, 2), "R")),
]
_PASS_OFF = []              # (offset_j0, delta) in plane elems
for ((t0, _), (t1, _)) in _PASS_TAPS:
    o0 = t0[0] * PW + t0[1]
    o1 = t1[0] * PW + t1[1]
    _PASS_OFF.append((o0, o1 - o0))


def _wap(base_ap, dims, offset):
    ap = base_ap.copy()
    ap.ap = bass_rust.VecI64Pair([list(base_ap.ap[0])] + [list(d) for d in dims])
    ap.offset = offset
    return ap


def _build(steps=T):
    nc = bacc.Bacc("TRN2", target_bir_lowering=False, debug=False)
    x_d = nc.dram_tensor("xin", [BL, T, 64, PH, PW], F8, kind="ExternalInput")
    w_d = nc.dram_tensor("win", [128, 2 * 2 * NPASS * 2 * 128], F8,
                         kind="ExternalInput")
    b_d = nc.dram_tensor("bin", [128, 4], F32, kind="ExternalInput")
    o_d = nc.dram_tensor("out", [BL, HID, H, W], F32, kind="ExternalOutput")

    with tile.TileContext(nc) as tc:
        with ExitStack() as ctx:
            const = ctx.enter_context(tc.tile_pool(name="const", bufs=1))
            psum = ctx.enter_context(tc.tile_pool(name="psum", bufs=2, space="PSUM"))
            gp = ctx.enter_context(tc.tile_pool(name="gates", bufs=2))

            wsb = const.tile([128, 2, 2, NPASS, 2, 128], F8, tag="wsb")
            nc.sync.dma_start(
                out=wsb[:, :, :, :, :, :],
                in_=w_d[:, :].rearrange("p (a b c d e) -> p a b c d e",
                                        a=2, b=2, c=NPASS, d=2),
            )
            bsb = const.tile([128, 4], F32, tag="bsb")
            nc.sync.dma_start(out=bsb[:, :], in_=b_d[:, :])

            # plane tiles: [img][parity]; img0: x@0:32, h@64:128; img1 mirror
            TT = [[const.tile([128, PH, PW], F8, tag=f"T{i}{p}", name=f"T{i}{p}")
                   for p in range(2)] for i in range(BL)]
            for i in range(BL):
                for p in range(2):
                    nc.vector.memset(TT[i][p][:, :, :], 0.0)
            # c state, packed img1 on 0:64 / img0 on 64:128
            C = const.tile([128, 2, HFREE], F16, tag="C")
            nc.vector.memset(C[:, :, :], 0.0)

            xpart = [(0, 64), (64, 128)]  # x8+xr partition range per img

            # prefetch x for t=0
            for i in range(BL):
                a, b = xpart[i]
                nc.sync.dma_start(out=TT[i][0][a:b, :, :], in_=x_d[i, 0, :, :, :])

            for t in range(steps):
                pi, po = t % 2, (t + 1) % 2
                if t + 1 < steps:
                    for i in range(BL):
                        a, b = xpart[i]
                        nc.sync.dma_start(out=TT[i][po][a:b, :, :],
                                          in_=x_d[i, t + 1, :, :, :])

                for half in range(2):
                    sg = []  # sif0, tgo0, sif1, tgo1
                    deferred = []
                    acts = []
                    for i in range(BL):
                        base = TT[i][pi][:, 0, 0:64]
                        for ci in range(2):
                            ps = psum.tile([128, 4, 512], F32, tag="ps",
                                           name=f"ps{t}_{half}_{i}_{ci}")
                            for rgl in range(4):
                                y0 = half * HROWS + rgl * RG
                                for p in range(NPASS):
                                    if half == 0 and rgl == 3 and p >= 3:
                                        # these read the first row written by
                                        # the previous step's half-1 chain;
                                        # emit them last so the PE can start
                                        # the step before that chain retires
                                        deferred.append((ps, i, ci, y0, p))
                                        continue
                                    o0, dlt = _PASS_OFF[p]
                                    rhs = _wap(
                                        base,
                                        [[dlt, 2], [PW, RG], [1, 64]],
                                        y0 * PW + o0,
                                    )
                                    nc.tensor.matmul(
                                        out=ps[:, rgl, :],
                                        lhsT=wsb[:, i, ci, p, :, :],
                                        rhs=rhs,
                                        start=(p == 0),
                                        stop=(p == NPASS - 1),
                                        perf_mode=DR,
                                    )
                            acts.append((ps, i, ci))
                            sg.append(None)
                        for ps, i2, ci, y0, p in deferred:
                            o0, dlt = _PASS_OFF[p]
                            rhs = _wap(
                                TT[i2][pi][:, 0, 0:64],
                                [[dlt, 2], [PW, RG], [1, 64]],
                                y0 * PW + o0,
                            )
                            nc.tensor.matmul(
                                out=ps[:, 3, :],
                                lhsT=wsb[:, i2, ci, p, :, :],
                                rhs=rhs,
                                start=(p == 0),
                                stop=(p == NPASS - 1),
                                perf_mode=DR,
                            )
                        deferred = []
                        for ps, i2, ci in acts:
                            g = gp.tile([128, HFREE], F16, tag=f"g{i2}{ci}",
                                        name=f"g{t}_{half}_{i2}_{ci}")
                            nc.scalar.activation(
                                out=g[:, :],
                                in_=ps[:, :, :],
                                func=(AF.Sigmoid if ci == 0 else AF.Tanh),
                                bias=bsb[:, 2 * i2 + ci : 2 * i2 + ci + 1],
                                scale=1.0 / SCL,
                            )
                            sg[2 * i2 + ci] = g
                        acts = []
                    sif0, tgo0, sif1, tgo1 = sg
                    # layout: sif0=[i0;f0] tgo0=[g0;o'0] sif1=[f1;i1]
                    # tgo1=[g1;o'1]

                    # early bridge: i1 (hi) down to lo  [Pool-queue DMA]
                    I1b = gp.tile([128, HFREE], F16, tag="I1b",
                                  name=f"I{t}_{half}")
                    nc.gpsimd.dma_start(out=I1b[0:64], in_=sif1[64:128])
                    # m2 = f*c (img0 on hi, img1 on lo) - after chunkA acts
                    M2 = gp.tile([128, HFREE], F16, tag="M2", name=f"M{t}_{half}")
                    nc.vector.tensor_mul(M2[64:128], sif0[64:128],
                                         C[64:128, half, :])
                    nc.vector.tensor_mul(M2[0:64], sif1[0:64], C[0:64, half, :])
                    # u1a = i0*g0 on lo, bridged up  [mid-half]
                    U = gp.tile([128, HFREE], F16, tag="U", name=f"U{t}_{half}")
                    U2 = gp.tile([128, HFREE], F16, tag="U2", name=f"U2{t}_{half}")
                    nc.vector.tensor_mul(U[0:64], sif0[0:64], tgo0[0:64])
                    nc.gpsimd.dma_start(out=U2[64:128], in_=U[0:64])
                    # late bridge: o'1 (hi) down to lo, concurrent with tail
                    O1b = gp.tile([128, HFREE], F16, tag="O1b",
                                  name=f"O{t}_{half}")
                    nc.gpsimd.dma_start(out=O1b[0:64], in_=tgo1[64:128])
                    # tail: u1b = i1*g1 on lo (both native after bridge)
                    nc.vector.tensor_mul(U2[0:64], I1b[0:64], tgo1[0:64])

                    Tt = gp.tile([128, HFREE], F16, tag="Tt", name=f"Tt{t}_{half}")
                    QF = HFREE // 2
                    for q in range(2):
                        qs = slice(q * QF, (q + 1) * QF)
                        # c' = f*c + i*g ; tct = tanh(c')
                        nc.vector.tensor_add(C[:, half, qs], M2[:, qs],
                                             U2[:, qs])
                        nc.scalar.activation(out=Tt[:, qs], in_=C[:, half, qs],
                                             func=AF.Tanh)
                        if t < steps - 1:
                            r0 = 1 + half * HROWS + q * (HROWS // 2)
                            qr = HROWS // 2
                            # v = 2h = (o'+1)*tct -> fp8 plane interiors
                            nc.vector.scalar_tensor_tensor(
                                out=TT[0][po][64:128, r0 : r0 + qr, 1 : 1 + W],
                                in0=tgo0[64:128, qs], scalar=1.0,
                                in1=Tt[64:128, qs],
                                op0=ALU.add, op1=ALU.mult)
                            nc.vector.scalar_tensor_tensor(
                                out=TT[1][po][0:64, r0 : r0 + qr, 1 : 1 + W],
                                in0=O1b[0:64, qs], scalar=1.0,
                                in1=Tt[0:64, qs],
                                op0=ALU.add, op1=ALU.mult)
                    if t == steps - 1:
                        # p = o' * tct
                        Pt = gp.tile([128, HFREE], F16, tag="Pt",
                                     name=f"P{t}_{half}")
                        nc.vector.tensor_mul(Pt[64:128], tgo0[64:128],
                                             Tt[64:128])
                        nc.vector.tensor_mul(Pt[0:64], O1b[0:64], Tt[0:64])
                        v16 = gp.tile([128, HFREE], F16, tag="v16",
                                      name=f"v{t}_{half}")
                        nc.vector.tensor_add(v16[:, :], Pt[:, :], Tt[:, :])
                        lk = gp.tile([128, HFREE], F16, tag="lk",
                                     name=f"l{t}_{half}")
                        nc.vector.scalar_tensor_tensor(
                            out=lk[:, :], in0=v16[:, :], scalar=0.01,
                            in1=v16[:, :], op0=ALU.mult, op1=ALU.max)
                        o32 = gp.tile([128, HFREE], F32, tag="o32",
                                      name=f"o{t}_{half}")
                        nc.scalar.activation(out=o32[:, :], in_=lk[:, :],
                                             func=AF.Copy, scale=0.5)
                        y0 = half * HROWS
                        nc.sync.dma_start(
                            out=o_d[0, :, y0 : y0 + HROWS, :],
                            in_=o32[64:128, :].rearrange(
                                "p (r c) -> p r c", c=W))
                        nc.sync.dma_start(
                            out=o_d[1, :, y0 : y0 + HROWS, :],
                            in_=o32[0:64, :].rearrange(
                                "p (r c) -> p r c", c=W))
    nc.compile()
    return nc


def _q8(a):
    return a.astype(ml_dtypes.float8_e4m3fn).astype(np.float32)


def _prep_weights(Wf, bf):
    Wp = np.asarray(Wf, np.float32).copy()   # [256, 96, 3, 3]; gates i,f,o,g
    bp = np.asarray(bf, np.float32).copy()
    Wp[128:192] *= 0.5                       # o' = tanh(o/2)
    bo = bp.copy()
    bo[128:192] *= 0.5
    Wp[:, CIN:] *= 0.5                       # h fed as v = 2h
    A = _q8(Wp * SCL)                        # base e4m3 weights (dequantized)
    R = Wp * SCL - A                         # residual rows
    Bx = Wp[:, 0:CIN] * SCL / XRS            # xr-plane weights

    colmap = {
        (0, 0): np.r_[0:64, 64:128],         # img0 chunkA = [i; f]
        (0, 1): np.r_[192:256, 128:192],     # img0 chunkB = [g; o']
        (1, 0): np.r_[64:128, 0:64],         # img1 chunkA = [f; i]
        (1, 1): np.r_[192:256, 128:192],     # img1 chunkB = [g; o']
    }
    wl = np.zeros((128, 2, 2, NPASS, 2, 128), NP8)
    bl = np.zeros((128, 4), np.float32)
    for i in range(2):
        for ci in range(2):
            rows = colmap[(i, ci)]
            bl[:, 2 * i + ci] = bo[rows]
            for p in range(NPASS):
                for j in range(2):
                    (ky, kx), kind = _PASS_TAPS[p][j]
                    if kind == "A":
                        wx = A[rows, 0:CIN, ky, kx].T        # [32, 128]
                        wh = A[rows, CIN:, ky, kx].T         # [64, 128]
                        wxr = Bx[rows, :, ky, kx].T          # [32, 128]
                    else:
                        wx = R[rows, 0:CIN, ky, kx].T
                        wh = R[rows, CIN:, ky, kx].T
                        wxr = None
                    if i == 0:
                        wl[0:32, i, ci, p, j, :] = wx
                        if wxr is not None:
                            wl[32:64, i, ci, p, j, :] = wxr
                        wl[64:128, i, ci, p, j, :] = wh
                    else:
                        wl[0:64, i, ci, p, j, :] = wh
                        wl[64:96, i, ci, p, j, :] = wx
                        if wxr is not None:
                            wl[96:128, i, ci, p, j, :] = wxr
    return np.ascontiguousarray(wl.reshape(128, -1)), np.ascontiguousarray(bl)


def _prep_x(x):
    # [B, T, 32, H, W] f32 -> [B, T, 64, PH, PW] fp8: ch 0:32 = q8(x) padded,
    # ch 32:64 = q8(XRS*(x - q8(x))) padded
    xf = np.asarray(x, np.float32)
    x8 = _q8(xf)
    xr = XRS * (xf - x8)
    xq = np.zeros((B, T, 64, PH, PW), NP8)
    xq[:, :, 0:CIN, 1 : 1 + H, 1 : 1 + W] = x8
    xq[:, :, CIN:, 1 : 1 + H, 1 : 1 + W] = xr
    return xq


_NC_CACHE = {}


def _get_nc():
    if "nc" not in _NC_CACHE:
        _NC_CACHE["nc"] = _build()
    return _NC_CACHE["nc"]


def _in_maps(x, Wf, bf):
    wl, bl = _prep_weights(Wf, bf)
    xq = _prep_x(x)
    return [
        {
            "xin": np.ascontiguousarray(xq[c * BL : (c + 1) * BL]),
            "win": wl,
            "bin": bl,
        }
        for c in range(NCORES)
    ]


def _run(x, W, b, trace=False, **spmd_kwargs):
    nc = _get_nc()
    res = run_bass_kernel_spmd(
        nc, _in_maps(x, W, b), core_ids=list(range(NCORES)), trace=trace,
        **spmd_kwargs,
    )
    out = np.concatenate([res.results[i]["out"] for i in range(NCORES)], axis=0)
    return np.ascontiguousarray(out, dtype=np.float32), res


def kernel(x, W, b):
    out, _ = _run(x, W, b)
    return out


# revision 18
# speedup vs baseline: 1.6314x; 1.1156x over previous
"""ConvLSTM block (B=16, T=16, 32->64ch, 64x64, 3x3 SAME) on 8 TRN2 cores.

Data-parallel over batch (2 images/core). The conv is computed with fp8-e4m3
DoubleRow matmuls (cost-model rate 0.5 cyc/row, K<=256/pass): per output tile
5 DR passes cover all 864 contraction rows via 9 "tap slots" — each slot
reads the SAME stored planes at a different (ky,kx) window offset, so h needs
no shifted copies at all.  Plane tile T[img] = [128, 66, 66] fp8: img0 has
x-channels on partitions 0:32 and h on 64:128 (img1 mirrored: h on 0:64) so
the next step's h can be written in-place by the vector engines.  x arrives
pre-quantized/padded from the host.  Weights are scaled by S=64 and
quantized to e4m3 on the host; the activation instruction un-scales.

Gate math: img0 chunkA=[i;f] (one sigmoid per 4-rg group), chunkB=[g;o'],
o'=tanh(o/2) with o-rows pre-halved; img1 uses swapped chunks ([f;i],[o';g])
so the c/h elementwise chain runs with both images packed on complementary
partition halves wherever the dataflow allows.  h is fed back as v=2h=
(o'+1)*tanh(c) with W h-columns pre-halved.
"""

from contextlib import ExitStack

import numpy as np
import ml_dtypes
import bass_rust

import concourse.mybir as mybir
import concourse.tile as tile
from concourse import bacc
from concourse.bass_utils import run_bass_kernel_spmd

F32 = mybir.dt.float32
F16 = mybir.dt.float16
F8 = mybir.dt.float8e4
AF = mybir.ActivationFunctionType
ALU = mybir.AluOpType
DR = mybir.MatmulPerfMode.DoubleRow

NP8 = ml_dtypes.float8_e4m3  # container dtype bass2jax expects for F8

# Problem shapes (hardcoded per harness contract).
B, T, CIN, HID, H, W = 16, 16, 32, 64, 64, 64
NCORES = 8
BL = B // NCORES            # images per core
PH = PW = 66                # padded plane
SCL = 64.0                  # weight scale (e4m3 range)
XRS = 64.0                  # x-residual plane scale
NPASS = 9
RG = 8                      # rows per PSUM bank tile
HROWS = 32                  # rows per half-group (4 rg)
HFREE = HROWS * W           # 2048

# 9 DR passes; HW requires ktile strides that are multiples of the row pitch,
# so ktiles pair taps within a column.  Each entry: ((tap_j0, kind_j0),
# (tap_j1, kind_j1)) with kind "A" = base weights, "R" = residual weights.
_PASS_TAPS = [
    (((0, 0), "A"), ((1, 0), "A")), (((0, 1), "A"), ((1, 1), "A")),
    (((0, 2), "A"), ((1, 2), "A")),
    (((1, 0), "R"), ((2, 0), "A")), (((1, 1), "R"), ((2, 1), "A")),
    (((1, 2), "R"), ((2, 2), "A")),
    (((0, 0), "R"), ((2, 0), "R")), (((0, 1), "R"), ((2, 1), "R")),
    (((0, 2), "R"), ((2, 2), "R")),
]
_PASS_OFF = []              # (offset_j0, delta) in plane elems
for ((t0, _), (t1, _)) in _PASS_TAPS:
    o0 = t0[0] * PW + t0[1]
    o1 = t1[0] * PW + t1[1]
    _PASS_OFF.append((o0, o1 - o0))


def _wap(base_ap, dims, offset):
    ap = base_ap.copy()
    ap.ap = bass_rust.VecI64Pair([list(base_ap.ap[0])] + [list(d) for d in dims])
    ap.offset = offset
    return ap


def _build(steps=T):
    nc = bacc.Bacc("TRN2", target_bir_lowering=False, debug=False)
    x_d = nc.dram_tensor("xin", [BL, T, 64, PH, PW], F8, kind="ExternalInput")
    w_d = nc.dram_tensor("win", [128, 2 * 2 * NPASS * 2 * 128], F8,
                         kind="ExternalInput")
    b_d = nc.dram_tensor("bin", [128, 4], F32, kind="ExternalInput")
    o_d = nc.dram_tensor("out", [BL, HID, H, W], F32, kind="ExternalOutput")

    with tile.TileContext(nc) as tc:
        with ExitStack() as ctx:
            const = ctx.enter_context(tc.tile_pool(name="const", bufs=1))
            psum = ctx.enter_context(tc.tile_pool(name="psum", bufs=2, space="PSUM"))
            gp = ctx.enter_context(tc.tile_pool(name="gates", bufs=2))

            wsb = const.tile([128, 2, 2, NPASS, 2, 128], F8, tag="wsb")
            nc.sync.dma_start(
                out=wsb[:, :, :, :, :, :],
                in_=w_d[:, :].rearrange("p (a b c d e) -> p a b c d e",
                                        a=2, b=2, c=NPASS, d=2),
            )
            bsb = const.tile([128, 4], F32, tag="bsb")
            nc.sync.dma_start(out=bsb[:, :], in_=b_d[:, :])

            # plane tiles: [img][parity]; img0: x8@0:32, xr@32:64, h@64:128;
            # img1 mirrored (h@0:64, x8@64:96, xr@96:128).  Only the h region
            # needs zeroing (initial state + pads); x regions are fully
            # DMA-written from host-padded planes.
            TT = [[const.tile([128, PH, PW], F8, tag=f"T{i}{p}", name=f"T{i}{p}")
                   for p in range(2)] for i in range(BL)]
            for p in range(2):
                nc.vector.memset(TT[0][p][64:128, :, :], 0.0)
                nc.gpsimd.memset(TT[1][p][0:64, :, :], 0.0)
            # c state, packed img1 on 0:64 / img0 on 64:128
            C = const.tile([128, 2, HFREE], F16, tag="C")
            nc.vector.memset(C[:, :, :], 0.0)

            xpart = [(0, 64), (64, 128)]  # x8+xr partition range per img

            # prefetch x for t=0
            for i in range(BL):
                a, b = xpart[i]
                nc.sync.dma_start(out=TT[i][0][a:b, :, :], in_=x_d[i, 0, :, :, :])

            for t in range(steps):
                pi, po = t % 2, (t + 1) % 2
                if t + 1 < steps:
                    for i in range(BL):
                        a, b = xpart[i]
                        nc.sync.dma_start(out=TT[i][po][a:b, :, :],
                                          in_=x_d[i, t + 1, :, :, :])

                for half in range(2):
                    # gate tiles, written unit-by-unit by the activations
                    sg = [gp.tile([128, HFREE], F16, tag=f"g{i}{ci}",
                                  name=f"g{t}_{half}_{i}_{ci}")
                          for i in range(BL) for ci in range(2)]
                    # PSUM units of 2 row-groups (1024 px) for fine recycling
                    for u in range(2):
                        deferred = []
                        acts = []
                        for i in range(BL):
                            base = TT[i][pi][:, 0, 0:64]
                            for ci in range(2):
                                ps = psum.tile([128, 2, 512], F32, tag="ps",
                                               name=f"ps{t}_{half}_{u}_{i}_{ci}",
                                               bufs=4)
                                for rgl in range(2):
                                    rg = u * 2 + rgl
                                    y0 = half * HROWS + rg * RG
                                    for p in range(NPASS):
                                        if half == 0 and rg == 3 and p >= 3:
                                            # reads the first row written by
                                            # the previous step's half-1
                                            # chain; emit last so the PE can
                                            # start before that chain retires
                                            deferred.append((ps, i, ci, y0, p))
                                            continue
                                        o0, dlt = _PASS_OFF[p]
                                        rhs = _wap(
                                            base,
                                            [[dlt, 2], [PW, RG], [1, 64]],
                                            y0 * PW + o0,
                                        )
                                        nc.tensor.matmul(
                                            out=ps[:, rgl, :],
                                            lhsT=wsb[:, i, ci, p, :, :],
                                            rhs=rhs,
                                            start=(p == 0),
                                            stop=(p == NPASS - 1),
                                            perf_mode=DR,
                                        )
                                acts.append((ps, i, ci))
                        for ps, i2, ci, y0, p in deferred:
                            o0, dlt = _PASS_OFF[p]
                            rhs = _wap(
                                TT[i2][pi][:, 0, 0:64],
                                [[dlt, 2], [PW, RG], [1, 64]],
                                y0 * PW + o0,
                            )
                            nc.tensor.matmul(
                                out=ps[:, 1, :],
                                lhsT=wsb[:, i2, ci, p, :, :],
                                rhs=rhs,
                                start=(p == 0),
                                stop=(p == NPASS - 1),
                                perf_mode=DR,
                            )
                        us = slice(u * 1024, (u + 1) * 1024)
                        for ps, i2, ci in acts:
                            nc.scalar.activation(
                                out=sg[2 * i2 + ci][:, us],
                                in_=ps[:, :, :],
                                func=(AF.Sigmoid if ci == 0 else AF.Tanh),
                                bias=bsb[:, 2 * i2 + ci : 2 * i2 + ci + 1],
                                scale=1.0 / SCL,
                            )
                    sif0, tgo0, sif1, tgo1 = sg
                    # layout: sif0=[i0;f0] tgo0=[g0;o'0] sif1=[f1;i1]
                    # tgo1=[g1;o'1]

                    # early bridge: i1 (hi) down to lo  [Pool-queue DMA]
                    I1b = gp.tile([128, HFREE], F16, tag="I1b",
                                  name=f"I{t}_{half}")
                    nc.gpsimd.dma_start(out=I1b[0:64], in_=sif1[64:128])
                    # m2 = f*c (img0 on hi, img1 on lo) - after chunkA acts
                    M2 = gp.tile([128, HFREE], F16, tag="M2", name=f"M{t}_{half}")
                    nc.vector.tensor_mul(M2[64:128], sif0[64:128],
                                         C[64:128, half, :])
                    nc.vector.tensor_mul(M2[0:64], sif1[0:64], C[0:64, half, :])
                    # u1a = i0*g0 on lo, bridged up  [mid-half]
                    U = gp.tile([128, HFREE], F16, tag="U", name=f"U{t}_{half}")
                    U2 = gp.tile([128, HFREE], F16, tag="U2", name=f"U2{t}_{half}")
                    nc.vector.tensor_mul(U[0:64], sif0[0:64], tgo0[0:64])
                    nc.gpsimd.dma_start(out=U2[64:128], in_=U[0:64])
                    # late bridge: o'1 (hi) down to lo, concurrent with tail
                    O1b = gp.tile([128, HFREE], F16, tag="O1b",
                                  name=f"O{t}_{half}")
                    nc.gpsimd.dma_start(out=O1b[0:64], in_=tgo1[64:128])
                    # tail: u1b = i1*g1 on lo (both native after bridge)
                    nc.vector.tensor_mul(U2[0:64], I1b[0:64], tgo1[0:64])

                    Tt = gp.tile([128, HFREE], F16, tag="Tt", name=f"Tt{t}_{half}")
                    QF = HFREE // 2
                    for q in range(2):
                        qs = slice(q * QF, (q + 1) * QF)
                        # c' = f*c + i*g ; tct = tanh(c')
                        nc.vector.tensor_add(C[:, half, qs], M2[:, qs],
                                             U2[:, qs])
                        nc.scalar.activation(out=Tt[:, qs], in_=C[:, half, qs],
                                             func=AF.Tanh)
                        if t < steps - 1:
                            r0 = 1 + half * HROWS + q * (HROWS // 2)
                            qr = HROWS // 2
                            # v = 2h = (o'+1)*tct -> fp8 plane interiors
                            nc.vector.scalar_tensor_tensor(
                                out=TT[0][po][64:128, r0 : r0 + qr, 1 : 1 + W],
                                in0=tgo0[64:128, qs], scalar=1.0,
                                in1=Tt[64:128, qs],
                                op0=ALU.add, op1=ALU.mult)
                            nc.vector.scalar_tensor_tensor(
                                out=TT[1][po][0:64, r0 : r0 + qr, 1 : 1 + W],
                                in0=O1b[0:64, qs], scalar=1.0,
                                in1=Tt[0:64, qs],
                                op0=ALU.add, op1=ALU.mult)
                    if t == steps - 1:
                        # p = o' * tct
                        Pt = gp.tile([128, HFREE], F16, tag="Pt",
                                     name=f"P{t}_{half}")
                        nc.vector.tensor_mul(Pt[64:128], tgo0[64:128],
                                             Tt[64:128])
                        nc.vector.tensor_mul(Pt[0:64], O1b[0:64], Tt[0:64])
                        v16 = gp.tile([128, HFREE], F16, tag="v16",
                                      name=f"v{t}_{half}")
                        nc.vector.tensor_add(v16[:, :], Pt[:, :], Tt[:, :])
                        lk = gp.tile([128, HFREE], F16, tag="lk",
                                     name=f"l{t}_{half}")
                        nc.vector.scalar_tensor_tensor(
                            out=lk[:, :], in0=v16[:, :], scalar=0.01,
                            in1=v16[:, :], op0=ALU.mult, op1=ALU.max)
                        o32 = gp.tile([128, HFREE], F32, tag="o32",
                                      name=f"o{t}_{half}")
                        nc.scalar.activation(out=o32[:, :], in_=lk[:, :],
                                             func=AF.Copy, scale=0.5)
                        y0 = half * HROWS
                        nc.sync.dma_start(
                            out=o_d[0, :, y0 : y0 + HROWS, :],
                            in_=o32[64:128, :].rearrange(
                                "p (r c) -> p r c", c=W))
                        nc.sync.dma_start(
                            out=o_d[1, :, y0 : y0 + HROWS, :],
                            in_=o32[0:64, :].rearrange(
                                "p (r c) -> p r c", c=W))
    nc.compile()
    return nc


def _q8(a):
    return a.astype(ml_dtypes.float8_e4m3fn).astype(np.float32)


def _prep_weights(Wf, bf):
    Wp = np.asarray(Wf, np.float32).copy()   # [256, 96, 3, 3]; gates i,f,o,g
    bp = np.asarray(bf, np.float32).copy()
    Wp[128:192] *= 0.5                       # o' = tanh(o/2)
    bo = bp.copy()
    bo[128:192] *= 0.5
    Wp[:, CIN:] *= 0.5                       # h fed as v = 2h
    A = _q8(Wp * SCL)                        # base e4m3 weights (dequantized)
    R = Wp * SCL - A                         # residual rows
    Bx = Wp[:, 0:CIN] * SCL / XRS            # xr-plane weights

    colmap = {
        (0, 0): np.r_[0:64, 64:128],         # img0 chunkA = [i; f]
        (0, 1): np.r_[192:256, 128:192],     # img0 chunkB = [g; o']
        (1, 0): np.r_[64:128, 0:64],         # img1 chunkA = [f; i]
        (1, 1): np.r_[192:256, 128:192],     # img1 chunkB = [g; o']
    }
    wl = np.zeros((128, 2, 2, NPASS, 2, 128), NP8)
    bl = np.zeros((128, 4), np.float32)
    for i in range(2):
        for ci in range(2):
            rows = colmap[(i, ci)]
            bl[:, 2 * i + ci] = bo[rows]
            for p in range(NPASS):
                for j in range(2):
                    (ky, kx), kind = _PASS_TAPS[p][j]
                    if kind == "A":
                        wx = A[rows, 0:CIN, ky, kx].T        # [32, 128]
                        wh = A[rows, CIN:, ky, kx].T         # [64, 128]
                        wxr = Bx[rows, :, ky, kx].T          # [32, 128]
                    else:
                        wx = R[rows, 0:CIN, ky, kx].T
                        wh = R[rows, CIN:, ky, kx].T
                        wxr = None
                    if i == 0:
                        wl[0:32, i, ci, p, j, :] = wx
                        if wxr is not None:
                            wl[32:64, i, ci, p, j, :] = wxr
                        wl[64:128, i, ci, p, j, :] = wh
                    else:
                        wl[0:64, i, ci, p, j, :] = wh
                        wl[64:96, i, ci, p, j, :] = wx
                        if wxr is not None:
                            wl[96:128, i, ci, p, j, :] = wxr
    return np.ascontiguousarray(wl.reshape(128, -1)), np.ascontiguousarray(bl)


def _prep_x(x):
    # [B, T, 32, H, W] f32 -> [B, T, 64, PH, PW] fp8: ch 0:32 = q8(x) padded,
    # ch 32:64 = q8(XRS*(x - q8(x))) padded
    xf = np.asarray(x, np.float32)
    x8 = _q8(xf)
    xr = XRS * (xf - x8)
    xq = np.zeros((B, T, 64, PH, PW), NP8)
    xq[:, :, 0:CIN, 1 : 1 + H, 1 : 1 + W] = x8
    xq[:, :, CIN:, 1 : 1 + H, 1 : 1 + W] = xr
    return xq


_NC_CACHE = {}


def _get_nc():
    if "nc" not in _NC_CACHE:
        _NC_CACHE["nc"] = _build()
    return _NC_CACHE["nc"]


def _in_maps(x, Wf, bf):
    wl, bl = _prep_weights(Wf, bf)
    xq = _prep_x(x)
    return [
        {
            "xin": np.ascontiguousarray(xq[c * BL : (c + 1) * BL]),
            "win": wl,
            "bin": bl,
        }
        for c in range(NCORES)
    ]


def _run(x, W, b, trace=False, **spmd_kwargs):
    nc = _get_nc()
    res = run_bass_kernel_spmd(
        nc, _in_maps(x, W, b), core_ids=list(range(NCORES)), trace=trace,
        **spmd_kwargs,
    )
    out = np.concatenate([res.results[i]["out"] for i in range(NCORES)], axis=0)
    return np.ascontiguousarray(out, dtype=np.float32), res


def kernel(x, W, b):
    out, _ = _run(x, W, b)
    return out
